# revision 15
# baseline (speedup 1.0000x reference)
"""Trainium2 Bass kernel for the 6-layer transformer LM (B=4, T=1024, E=1024,
H=16, V=32000) on 8 NeuronCores.

Strategy: data-parallel over tokens, single kernel launch for the whole model.
Each core owns 512 tokens (batch b = core//2; 4 interleaved 128-token blocks:
{0,3,4,7} for even cores, {1,2,5,6} for odd). All weights are replicated and
streamed from HBM. Attention needs q/v of the sibling core's tokens: after
LayerNorm the normalized hidden state h (bf16, 1MB) is exchanged with a
pair-wise device AllGather (~16us, hidden under the k projection); each core
then computes q/v for all 1024 tokens of its batch element (redundant +2
projections) and full causal attention for its own 512 query positions.
FFN, final LN and the LM head are purely local (per-token). Host does the
embedding gather up front and reassembles logits blocks at the end.

LayerNorm gains are folded into the following matmul's weights host-side;
lnf_b is folded into the LM-head bias. ln1_b/ln2_b must be zero (asserted).
Matmuls in bf16 (fp32 PSUM accumulation); softmax/stats in fp32.
"""
import os
import sys
sys.path.insert(0, "/opt/trn_rl_repo")

import numpy as np
import ml_dtypes

import concourse.bacc as bacc
import concourse.tile as tile
from concourse import mybir
from concourse.bass import ts, ds

P = 128
B, T, E, H, HD, V, L = 4, 1024, 1024, 16, 64, 32000, 6
KO = E // P            # 8 contraction chunks
TW = 512               # tokens per core
NB = T // P            # 8 token blocks per batch element
VB = V // P            # 250 vocab blocks
FF = 4 * E             # 4096
EPS = 1e-5
SCALE = HD ** -0.5
NMASK = -1.0e9

LOCAL_BLOCKS = [[0, 3, 4, 7], [1, 2, 5, 6]]   # per sub-core token-block sets
GO = [0, 3, 4, 7, 1, 2, 5, 6]                 # gathered (pair-concat) block order
POS = [GO.index(j) for j in range(8)]          # s-block j -> gathered position
SUF_W = [512, 512, 384, 384, 256, 256, 128, 128]  # union suffix widths
SUF_S = [0, 0, 128, 128, 256, 256, 384, 384]      # union suffix starts
# per s-block j, exactly one 128-col block (the first of the suffix) may need
# masking (diagonal or over-causal); it sits at packed t-slot SUF_S[j]//128.

DT = mybir.dt.bfloat16
F32 = mybir.dt.float32
AF = mybir.ActivationFunctionType
OP = mybir.AluOpType


def np_dt(dt):
    return ml_dtypes.bfloat16 if dt == mybir.dt.bfloat16 else np.float32


def build_model(n_layers=L):
    nc = bacc.Bacc("TRN2", target_bir_lowering=False, num_devices=8)
    n_wl = max(1, n_layers)

    x0_d = nc.dram_tensor("x0", [P, KO, TW], F32, kind="ExternalInput")
    wq_d = nc.dram_tensor("wq", [n_wl, P, KO, E], DT, kind="ExternalInput")
    wk_d = nc.dram_tensor("wk", [n_wl, P, KO, E], DT, kind="ExternalInput")
    wv_d = nc.dram_tensor("wv", [n_wl, P, KO, E], DT, kind="ExternalInput")
    wo_d = nc.dram_tensor("wo", [n_wl, P, KO, E], DT, kind="ExternalInput")
    w1_d = nc.dram_tensor("w1", [n_wl, P, KO, FF], DT, kind="ExternalInput")
    w2_d = nc.dram_tensor("w2", [n_wl, P, FF // P, E], DT, kind="ExternalInput")
    wlm_d = nc.dram_tensor("wlm", [P, KO, V], DT, kind="ExternalInput")
    mask_d = nc.dram_tensor("mask", [8, P, P], DT, kind="ExternalInput")
    bo_d = nc.dram_tensor("bo", [n_wl, P, KO], F32, kind="ExternalInput")
    b1_d = nc.dram_tensor("b1", [n_wl, P, FF // P], F32, kind="ExternalInput")
    b2_d = nc.dram_tensor("b2", [n_wl, P, KO], F32, kind="ExternalInput")
    blm_d = nc.dram_tensor("blm", [P, VB], F32, kind="ExternalInput")
    out_d = nc.dram_tensor("out", [VB, P, TW], DT, kind="ExternalOutput")
    xout_d = nc.dram_tensor("xout", [P, KO, TW], F32, kind="ExternalOutput")
    dbg = bool(int(os.environ.get("TRN_LLM_DBG", "0")))
    if dbg:
        hdump_d = nc.dram_tensor("hdump", [P, KO, 2, TW], DT, kind="ExternalOutput")
        qdump_d = nc.dram_tensor("qdump", [P, 8, 2, TW], DT, kind="ExternalOutput")
        kdump_d = nc.dram_tensor("kdump", [P, KO, TW], DT, kind="ExternalOutput")
        vdump_d = nc.dram_tensor("vdump", [P, 8, H, HD], DT, kind="ExternalOutput")
        adump_d = nc.dram_tensor("adump", [P, KO, TW], DT, kind="ExternalOutput")

    with tile.TileContext(nc) as tc:
        with (
            tc.tile_pool(name="cst", bufs=1) as cst,
            tc.tile_pool(name="xp", bufs=1) as xp,
            tc.tile_pool(name="lnp", bufs=1) as lnp,       # xb/att, xsq/k, h
            tc.tile_pool(name="bigp", bufs=1) as bigp,     # h_all / ffn act
            tc.tile_pool(name="qp", bufs=1) as qp,
            tc.tile_pool(name="vp", bufs=1) as vp,
            tc.tile_pool(name="ewp", bufs=3) as ewp,       # expw etc
            tc.tile_pool(name="smallp", bufs=1) as smallp,
            tc.tile_pool(name="biasp", bufs=2) as biasp,
            tc.tile_pool(name="outp", bufs=2) as outp,
            tc.tile_pool(name="dramp", bufs=2, space="DRAM") as dramp,
            tc.tile_pool(name="pp", bufs=3, space="PSUM") as pp,
            tc.tile_pool(name="pa", bufs=2, space="PSUM") as pa,
            tc.tile_pool(name="pbc", bufs=2, space="PSUM") as pbc,
        ):
            # ---- constants ----
            ones_col = cst.tile([P, 1], DT)
            nc.vector.memset(ones_col[:], 1.0)
            ones_row = cst.tile([1, P], DT)
            nc.vector.memset(ones_row[:], 1.0)
            eps_t = cst.tile([1, 1], F32)
            nc.vector.memset(eps_t[:], EPS)
            mask_sb = cst.tile([P, 8, P], DT)
            nc.sync.dma_start(mask_sb[:], mask_d.rearrange("j p c -> p j c"))
            blm_sb = cst.tile([P, VB], F32)
            nc.sync.dma_start(blm_sb[:], blm_d[:])

            # ---- residual stream (fp32, resident) ----
            x_sb = xp.tile([P, KO, TW], F32)
            nc.sync.dma_start(x_sb[:], x0_d[:])

            def ln_normalize(h_dst):
                """LN stats over the local x strip; writes normalized bf16 h
                (gain folded into the following weights host-side)."""
                xb = lnp.tile([P, KO, TW], DT, tag="xb", name="xb")
                nc.vector.tensor_copy(out=xb[:], in_=x_sb[:])
                xsq = lnp.tile([P, KO, TW], DT, tag="xsq", name="xsq")
                nc.scalar.activation(xsq[:], xb[:], AF.Square)
                ps_sum = pbc.tile([1, TW], F32, tag="pbc", name="ps_sum")
                ps_sq = pbc.tile([1, TW], F32, tag="pbc", name="ps_sq")
                for ko in range(KO):
                    nc.tensor.matmul(ps_sum[:], ones_col[:], xb[:, ko],
                                     start=(ko == 0), stop=(ko == KO - 1))
                for ko in range(KO):
                    nc.tensor.matmul(ps_sq[:], ones_col[:], xsq[:, ko],
                                     start=(ko == 0), stop=(ko == KO - 1))
                inv = 1.0 / E
                mean = smallp.tile([1, TW], F32, tag="stat", name="mean", bufs=3)
                nc.vector.tensor_scalar_mul(mean[:], ps_sum[:], inv)
                m_dt = smallp.tile([1, TW], DT, tag="m_dt", name="m_dt")
                nc.vector.tensor_copy(out=m_dt[:], in_=mean[:])
                var = smallp.tile([1, TW], F32, tag="stat", name="var", bufs=3)
                nc.vector.tensor_scalar_mul(var[:], ps_sq[:], inv)
                msq = smallp.tile([1, TW], F32, tag="stat", name="msq", bufs=3)
                nc.vector.tensor_mul(msq[:], mean[:], mean[:])
                nc.vector.tensor_sub(var[:], var[:], msq[:])
                std = smallp.tile([1, TW], F32, tag="stat", name="std", bufs=3)
                nc.scalar.activation(std[:], var[:], AF.Sqrt, bias=eps_t[:1])
                rstd = smallp.tile([1, TW], F32, tag="stat", name="rstd", bufs=3)
                nc.vector.reciprocal_approx_fast(out=rstd[:], in_=std[:])
                r_dt = smallp.tile([1, TW], DT, tag="r_dt", name="r_dt")
                nc.vector.tensor_copy(out=r_dt[:], in_=rstd[:])
                mb = pbc.tile([P, TW], F32, tag="pbc", name="mb")
                nc.tensor.matmul(mb[:], ones_row[:], m_dt[:], start=True, stop=True)
                rb = pbc.tile([P, TW], F32, tag="pbc", name="rb")
                nc.tensor.matmul(rb[:], ones_row[:], r_dt[:], start=True, stop=True)
                for ko in range(KO):
                    ntmp = ewp.tile([P, TW], DT, tag="ntmp", name="ntmp")
                    nc.vector.tensor_sub(ntmp[:], xb[:, ko], mb[:])
                    nc.vector.tensor_mul(h_dst[:, ko], ntmp[:], rb[:])

            def layer_body(l, wap, wfp, w2p):
                if True:
                    # ---- LN1 + pair AllGather of h ----
                    h_loc = lnp.tile([P, KO, TW], DT, tag="hln", name="h_loc")
                    ln_normalize(h_loc)
                    bounce = dramp.tile([P, KO, TW], DT, tag="bounce",
                                        name="bounce")
                    nc.sync.dma_start(bounce[:], h_loc[:])
                    gath = dramp.tile([2, P, KO, TW], DT, tag="gath", name="gath")
                    nc.gpsimd.collective_compute(
                        "AllGather", OP.bypass,
                        replica_groups=[[0, 1], [2, 3], [4, 5], [6, 7]],
                        ins=[bounce[:]], outs=[gath[:]],
                    )

                    # ---- k projection from local h (overlaps the AllGather) ----
                    wk_sb = wap.tile([P, KO, E], DT, tag="wa", name="wk_sb")
                    nc.sync.dma_start(wk_sb[:], wk_d[l])
                    k_sb = lnp.tile([P, KO, TW], DT, tag="xsq", name="k_sb")
                    for hp in range(8):
                        pk = pp.tile([P, TW], F32, tag="pp", name="pk")
                        for ko in range(KO):
                            nc.tensor.matmul(pk[:], wk_sb[:, ko, ts(hp, P)],
                                             h_loc[:, ko], start=(ko == 0),
                                             stop=(ko == KO - 1))
                        nc.vector.tensor_copy(out=k_sb[:, hp], in_=pk[:])

                    h_all = bigp.tile([P, KO, 2, TW], DT, tag="big", name="h_all")
                    for r in range(2):
                        nc.sync.dma_start(h_all[:, :, r], gath[r])

                    # ---- q projection (all 1024 tokens of the pair) ----
                    wq_sb = wap.tile([P, KO, E], DT, tag="wa", name="wq_sb")
                    nc.sync.dma_start(wq_sb[:], wq_d[l])
                    q_sb = qp.tile([P, 8, 2, TW], DT, name="q_sb")
                    for hp in range(8):
                        for r in range(2):
                            pq = pp.tile([P, TW], F32, tag="pp", name="pq")
                            for ko in range(KO):
                                nc.tensor.matmul(pq[:], wq_sb[:, ko, ts(hp, P)],
                                                 h_all[:, ko, r], start=(ko == 0),
                                                 stop=(ko == KO - 1))
                            nc.vector.tensor_copy(out=q_sb[:, hp, r], in_=pq[:])

                    # ---- v projection (token-major, with fused ones column) ----
                    wv_sb = wap.tile([P, KO, E], DT, tag="wa", name="wv_sb")
                    nc.sync.dma_start(wv_sb[:], wv_d[l])
                    v_sb = vp.tile([P, 8, H, HD], DT, name="v_sb")
                    for g in range(8):
                        r, tb = g // 4, g % 4
                        for vh in range(2):
                            pv = pp.tile([P, TW], F32, tag="pp", name="pv")
                            for ko in range(KO):
                                nc.tensor.matmul(
                                    pv[:], h_all[:, ko, r, ts(tb, P)],
                                    wv_sb[:, ko, ts(vh, TW)],
                                    start=(ko == 0), stop=(ko == KO - 1))
                            nc.vector.tensor_copy(
                                out=v_sb[:, g, vh * 8:(vh + 1) * 8, :],
                                in_=pv[:].rearrange("p (h d) -> p h d", h=8))

                    # ---- attention: per head, union causal suffixes ----
                    # All DVE/ACT ops keep identical partition offsets on every
                    # operand (lanes are partition-fixed); PE tile_position
                    # performs the cross-partition placements.
                    att_sb = lnp.tile([P, KO, TW], DT, tag="xb", name="att_sb")
                    for hp in range(8):
                        att_ps = pa.tile([P, TW], F32, tag="pa", name="att_ps")
                        dens = []
                        for hpar in range(2):
                            h = 2 * hp + hpar
                            hrow = 64 * hpar
                            den_ps = pbc.tile([1, TW], F32, tag="pbc",
                                              name="den_ps")
                            for j in range(8):
                                W, S = SUF_W[j], SUF_S[j]
                                psc = pp.tile([P, TW], F32, tag="pp", name="psc")
                                nc.tensor.matmul(
                                    psc[:, 0:W],
                                    q_sb[hrow:hrow + 64, hp, POS[j] // 4,
                                         ts(POS[j] % 4, P)],
                                    k_sb[hrow:hrow + 64, hp, S:S + W],
                                    start=True, stop=True,
                                    tile_position=(hrow, 0))
                                expw = ewp.tile([P, TW], DT, tag="expw",
                                                name="expw")
                                mtmp = ewp.tile([P, P], DT, tag="mtmp",
                                                name="mtmp")
                                nc.vector.tensor_add(mtmp[:], psc[:, 0:P],
                                                     mask_sb[:, j])
                                nc.scalar.activation(expw[:, 0:P], mtmp[:],
                                                     AF.Exp, scale=SCALE)
                                if W > P:
                                    nc.scalar.activation(expw[:, P:W],
                                                         psc[:, P:W],
                                                         AF.Exp, scale=SCALE)
                                nc.tensor.matmul(
                                    att_ps[hrow:hrow + 64, S:S + W],
                                    v_sb[:, POS[j], h], expw[:, 0:W],
                                    start=(j == 0), stop=(j == 7),
                                    tile_position=(0, hrow),
                                    skip_group_check=True)
                                nc.tensor.matmul(
                                    den_ps[0:1, S:S + W],
                                    ones_col[:], expw[:, 0:W],
                                    start=(j == 0), stop=(j == 7),
                                    skip_group_check=True)
                            dens.append(den_ps)
                        for hpar in range(2):
                            hrow = 64 * hpar
                            recip = smallp.tile([1, TW], F32, tag="recip",
                                                name="recip", bufs=2)
                            nc.vector.reciprocal_approx_fast(
                                out=recip[:], in_=dens[hpar][0:1, :])
                            recip_dt = smallp.tile([1, TW], DT, tag="recip_dt",
                                                   name="recip_dt", bufs=2)
                            nc.vector.tensor_copy(out=recip_dt[:], in_=recip[:])
                            rbc = pbc.tile([P, TW], F32, tag="pbc", name="rbc")
                            nc.tensor.matmul(rbc[hrow:hrow + 64, :],
                                             ones_row[:, 0:64], recip_dt[:],
                                             start=True, stop=True,
                                             tile_position=(0, hrow))
                            araw = ewp.tile([P, TW], DT, tag="araw",
                                            name="araw")
                            nc.scalar.activation(araw[hrow:hrow + 64, :],
                                                 att_ps[hrow:hrow + 64, :],
                                                 AF.Copy)
                            nc.vector.tensor_mul(att_sb[hrow:hrow + 64, hp],
                                                 araw[hrow:hrow + 64, :],
                                                 rbc[hrow:hrow + 64, :])

                    if dbg and l == 0:
                        nc.sync.dma_start(hdump_d[:], h_all[:])
                        nc.sync.dma_start(qdump_d[:], q_sb[:])
                        nc.sync.dma_start(kdump_d[:], k_sb[:])
                        nc.sync.dma_start(vdump_d[:], v_sb[:])
                        nc.sync.dma_start(adump_d[:], att_sb[:])

                    # ---- output projection + residual (+bo) ----
                    wo_sb = wap.tile([P, KO, E], DT, tag="wa", name="wo_sb")
                    nc.sync.dma_start(wo_sb[:], wo_d[l])
                    bo_sb = biasp.tile([P, KO], F32, tag="bo", name="bo_sb")
                    nc.sync.dma_start(bo_sb[:], bo_d[l])
                    for eb in range(KO):
                        po = pp.tile([P, TW], F32, tag="pp", name="po")
                        for hp in range(8):
                            nc.tensor.matmul(po[:], wo_sb[:, hp, ts(eb, P)],
                                             att_sb[:, hp], start=(hp == 0),
                                             stop=(hp == 7))
                        nc.vector.scalar_tensor_tensor(
                            out=x_sb[:, eb], in0=po[:],
                            scalar=bo_sb[:, eb:eb + 1],
                            in1=x_sb[:, eb], op0=OP.add, op1=OP.add)

                    # ---- FFN ----
                    h2 = lnp.tile([P, KO, TW], DT, tag="hln", name="h2")
                    ln_normalize(h2)
                    b1_sb = biasp.tile([P, FF // P], F32, tag="b1", name="b1_sb")
                    nc.sync.dma_start(b1_sb[:], b1_d[l])
                    a_sb = bigp.tile([P, FF // P, TW], DT, tag="big", name="a_sb")
                    for fc in range(8):
                        w1_sb = wfp.tile([P, KO, 512], DT, tag="wf", name="w1_sb")
                        nc.sync.dma_start(w1_sb[:], w1_d[l][:, :, ts(fc, 512)])
                        for fb in range(4):
                            f = fc * 4 + fb
                            pf = pp.tile([P, TW], F32, tag="pp", name="pf")
                            for ko in range(KO):
                                nc.tensor.matmul(pf[:], w1_sb[:, ko, ts(fb, P)],
                                                 h2[:, ko], start=(ko == 0),
                                                 stop=(ko == KO - 1))
                            nc.scalar.activation(a_sb[:, f], pf[:], AF.Relu,
                                                 bias=b1_sb[:, f:f + 1])
                    b2_sb = biasp.tile([P, KO], F32, tag="b2", name="b2_sb")
                    nc.sync.dma_start(b2_sb[:], b2_d[l])
                    for eb in range(KO):
                        po = pp.tile([P, TW], F32, tag="pp", name="po2")
                        for fh in range(2):
                            w2_sb = w2p.tile([P, 16, P], DT, tag="w2",
                                             name="w2_sb")
                            nc.sync.dma_start(
                                w2_sb[:], w2_d[l][:, ds(fh * 16, 16), ts(eb, P)])
                            for fo in range(16):
                                nc.tensor.matmul(po[:], w2_sb[:, fo],
                                                 a_sb[:, fh * 16 + fo],
                                                 start=(fh == 0 and fo == 0),
                                                 stop=(fh == 1 and fo == 15))
                        nc.vector.scalar_tensor_tensor(
                            out=x_sb[:, eb], in0=po[:],
                            scalar=b2_sb[:, eb:eb + 1],
                            in1=x_sb[:, eb], op0=OP.add, op1=OP.add)

            with (
                tc.tile_pool(name="wap", bufs=2) as wap,
                tc.tile_pool(name="wfp", bufs=2) as wfp,
                tc.tile_pool(name="w2p", bufs=2) as w2p,
            ):
                for l in range(n_layers):
                    layer_body(l, wap, wfp, w2p)

            nc.sync.dma_start(xout_d[:], x_sb[:])

            # ---- final LN + LM head (local tokens, full vocab) ----
            with tc.tile_pool(name="wlmp", bufs=3) as wlmp:
                hf = lnp.tile([P, KO, TW], DT, tag="hln", name="hf")
                ln_normalize(hf)
                off = 0
                while off < V:
                    cw = min(512, V - off)
                    wlm_sb = wlmp.tile([P, KO, 512], DT, tag="wlm", name="wlm_sb")
                    nc.sync.dma_start(wlm_sb[:, :, 0:cw], wlm_d[:, :, ds(off, cw)])
                    for vb in range(cw // P):
                        gvb = off // P + vb
                        plm = pp.tile([P, TW], F32, tag="pp", name="plm")
                        for ko in range(KO):
                            nc.tensor.matmul(plm[:], wlm_sb[:, ko, ts(vb, P)],
                                             hf[:, ko], start=(ko == 0),
                                             stop=(ko == KO - 1))
                        ob = outp.tile([P, TW], DT, tag="ob", name="ob")
                        nc.scalar.activation(ob[:], plm[:], AF.Identity,
                                             bias=blm_sb[:, gvb:gvb + 1])
                        nc.sync.dma_start(out_d[gvb], ob[:])
                    off += cw
    nc.compile()
    return nc


# ====================== host orchestration ======================

_program = None
last_exec_ns = 0
N_LAYERS = L          # debug knob: run only the first N layers


def _get_program():
    global _program
    if _program is None:
        _program = build_model(N_LAYERS)
    return _program


def _host_weights(inputs):
    """Fold LN gains, reshape weights to the device layouts (shared by cores)."""
    ndt = np_dt(DT)
    f32 = lambda k: np.asarray(inputs[k], dtype=np.float32)
    Wq, Wk, Wv, Wo = f32("Wq"), f32("Wk"), f32("Wv"), f32("Wo")
    W1, W2 = f32("W1"), f32("W2")
    ln1_g, ln1_b = f32("ln1_g"), f32("ln1_b")
    ln2_g, ln2_b = f32("ln2_g"), f32("ln2_b")
    lnf_g, lnf_b = f32("lnf_g"), f32("lnf_b")
    Wlm, blm = f32("Wlm"), f32("blm")
    assert np.all(ln1_b == 0) and np.all(ln2_b == 0), "nonzero ln betas unsupported"

    def proj(w):  # [L, E, M] -> [L, 128, KO, M]
        Lx, _, M = w.shape
        return np.ascontiguousarray(
            w.reshape(Lx, KO, P, M).transpose(0, 2, 1, 3)).astype(ndt)

    wq = proj(ln1_g[:, :, None] * Wq.reshape(L, E, E))
    wk = proj(ln1_g[:, :, None] * Wk.reshape(L, E, E))
    wv = proj(ln1_g[:, :, None] * Wv.reshape(L, E, E))
    wo = proj(Wo)
    w1 = proj(ln2_g[:, :, None] * W1)
    w2 = np.ascontiguousarray(
        W2.reshape(L, FF // P, P, E).transpose(0, 2, 1, 3)).astype(ndt)
    wlm = np.ascontiguousarray(
        (lnf_g[:, None] * Wlm).reshape(KO, P, V).transpose(1, 0, 2)).astype(ndt)
    blm_eff = (blm + lnf_b @ Wlm).astype(np.float32)
    blm_h = np.ascontiguousarray(blm_eff.reshape(VB, P).T)
    bo_h = np.ascontiguousarray(f32("bo").reshape(L, KO, P).transpose(0, 2, 1))
    b1_h = np.ascontiguousarray(
        f32("b1").reshape(L, FF // P, P).transpose(0, 2, 1)).astype(np.float32)
    b2_h = np.ascontiguousarray(f32("b2").reshape(L, KO, P).transpose(0, 2, 1))
    return dict(wq=wq, wk=wk, wv=wv, wo=wo, w1=w1, w2=w2, wlm=wlm,
                blm=blm_h, bo=bo_h, b1=b1_h, b2=b2_h)


def _host_masks(sub):
    """Per-core mask tiles [8, 128, 128] for the (single) maskable block of
    each s-block's union suffix."""
    lb = LOCAL_BLOCKS[sub]
    m = np.zeros((8, P, P), np.float32)
    for j in range(8):
        slot = SUF_S[j] // P
        g = lb[slot]
        if g > j:
            continue                      # fully allowed
        elif g < j:
            m[j, :, :] = NMASK            # over-causal: fully masked
        else:                             # diagonal: allow s_row <= t_col
            sr = np.arange(P)[:, None]
            tc_ = np.arange(P)[None, :]
            m[j] = np.where(sr <= tc_, 0.0, NMASK)
    return m.astype(np_dt(DT))


def kernel(**inputs):
    global last_exec_ns
    from concourse.bass_utils import run_bass_kernel_spmd
    trace = bool(int(os.environ.get("TRN_LLM_TRACE", "0")))

    idx = np.asarray(inputs["idx"])
    tok_emb = np.asarray(inputs["tok_emb"], dtype=np.float32)
    pos_emb = np.asarray(inputs["pos_emb"], dtype=np.float32)
    shared = _host_weights(inputs)
    masks = [_host_masks(0), _host_masks(1)]

    if N_LAYERS != L:                      # debug: truncate per-layer weights
        for kk in ("wq", "wk", "wv", "wo", "w1", "w2", "bo", "b1", "b2"):
            shared[kk] = np.ascontiguousarray(shared[kk][:max(1, N_LAYERS)])

    emb = tok_emb[idx] + pos_emb[None, :T]      # [B, T, E] fp32
    maps = []
    for c in range(8):
        b, sub = c // 2, c % 2
        lb = LOCAL_BLOCKS[sub]
        xs = np.concatenate([emb[b, g * P:(g + 1) * P] for g in lb], axis=0)
        x0 = np.ascontiguousarray(
            xs.T.reshape(KO, P, TW).transpose(1, 0, 2)).astype(np.float32)
        m = dict(shared)
        m["x0"] = x0
        m["mask"] = masks[sub]
        maps.append(m)

    nc = _get_program()
    res = run_bass_kernel_spmd(nc, maps, list(range(8)), trace=trace)
    last_exec_ns = res.exec_time_ns or 0
    global last_results
    last_results = res.results

    logits = np.empty((B, T, V), np.float32)
    for c in range(8):
        b, sub = c // 2, c % 2
        lb = LOCAL_BLOCKS[sub]
        out = np.asarray(res.results[c]["out"], dtype=np.float32)  # [VB,128,TW]
        out = out.transpose(2, 0, 1).reshape(TW, V)                # [TW, V]
        for p, g in enumerate(lb):
            logits[b, g * P:(g + 1) * P] = out[p * P:(p + 1) * P]
    return logits
